# revision 7
# baseline (speedup 1.0000x reference)
"""Trainium2 Bass kernel for nn_ContrastiveLoss (SimCLR-style, N=8192, D=128).

Sharding: rows of the NxN sim matrix split across 8 cores (1024 rows each).
Each core receives the full z = concat(emb0, emb1) ROTATED so its own rows
come first (np.roll(z, -core*1024, axis=0)).  With that rotation the diagonal
of row-block b sits at local columns [b*128, b*128+128) and the positive pair
at local columns [4096+b*128, ...), identical on every core -> one SPMD
program, no collectives.  Per-core output is a [128,1] partial sum of
(ln S_r - 10*sim_pos_r); the host combines: loss = 10 + sum(partials)/8192.

Math (per row r, fixed max = 1.0 since cosine sim <= 1):
  e_j  = exp(10*G_rj - 10),  S_r = sum_j e_j - e_rr
  lse_r = 10 + ln(S_r);  contrib_r = lse_r - 10*G_{r,pos(r)}
  loss  = mean_r(contrib_r)
"""

import sys

sys.path.insert(0, "/opt/trn_rl_repo")

from contextlib import ExitStack

import numpy as np

import concourse.bass as bass
import concourse.bacc as bacc
import concourse.tile as tile
from concourse import mybir
from concourse import bass_utils
from concourse.masks import make_identity

B = 4096
D = 128
N = 2 * B            # 8192 rows of z
NCORES = 8
ROWS = N // NCORES   # 1024 rows per core
NBLK = ROWS // 128   # 8 row-blocks per core
CHUNK = 2048         # psum tile width (4 banks)
NCHUNK = N // CHUNK  # 4 column chunks
SEG = 512            # matmul moving-operand max (fp32)
NTILE = N // 128     # 64 partition-tiles of z
INV_T = 10.0         # 1/temperature
EPS = 1e-8

F32 = mybir.dt.float32
AX = mybir.AxisListType
AF = mybir.ActivationFunctionType


def _build() -> bass.Bass:
    nc = bacc.Bacc(None)
    z_in = nc.declare_dram_parameter("z", [N, D], F32, isOutput=False)
    out = nc.declare_dram_parameter("partial", [128, 1], F32, isOutput=True)

    z_re = z_in.rearrange("(n p) d -> p n d", p=128)  # row = n*128 + p

    with tile.TileContext(nc) as tc:
        with ExitStack() as ctx:
            persist = ctx.enter_context(tc.tile_pool(name="persist", bufs=1))
            work = ctx.enter_context(tc.tile_pool(name="work", bufs=3))
            junkp = ctx.enter_context(tc.tile_pool(name="junk", bufs=2))
            psum = ctx.enter_context(tc.tile_pool(name="psum", bufs=2, space="PSUM"))

            ident = persist.tile([128, 128], F32)
            make_identity(nc, ident)
            # non-Copy activations need bias as an SBUF AP
            b_zero = persist.tile([128, 1], F32)
            nc.vector.memset(b_zero, 0.0)
            b_neg10 = persist.tile([128, 1], F32)
            nc.vector.memset(b_neg10, -INV_T)

            # ---- load z: [8192,128] -> SBUF [128p, 64, 128] -------------
            z_sb = persist.tile([128, NTILE, D], F32)
            DMA_G = 8  # tiles per DMA
            for i in range(NTILE // DMA_G):
                nc.sync.dma_start(
                    out=z_sb[:, i * DMA_G : (i + 1) * DMA_G, :],
                    in_=z_re[:, i * DMA_G : (i + 1) * DMA_G, :],
                )

            # ---- row norms ----------------------------------------------
            sq = persist.tile([128, NTILE, D], F32)
            sumsq = persist.tile([128, NTILE], F32)
            for i in range(NTILE // DMA_G):
                sl = slice(i * DMA_G, (i + 1) * DMA_G)
                nc.vector.tensor_mul(sq[:, sl, :], z_sb[:, sl, :], z_sb[:, sl, :])
                nc.vector.reduce_sum(sumsq[:, sl], sq[:, sl, :], axis=AX.X)
            rn = persist.tile([128, NTILE], F32)
            nc.scalar.activation(rn, sumsq, AF.Sqrt, bias=b_zero)          # norm
            nc.vector.tensor_scalar_max(rn, rn, EPS)          # clamp
            nc.vector.reciprocal(rn, rn)                      # 1/max(norm,eps)

            # ---- normalize + transpose into znT chunks [128d, 2048] -----
            znT = [
                persist.tile([128, CHUNK], F32, tag=f"znT{j}", name=f"znT{j}")
                for j in range(NCHUNK)
            ]
            for n in range(NTILE):
                znsc = work.tile([128, 128], F32, tag="znsc")
                nc.vector.tensor_scalar_mul(znsc, z_sb[:, n, :], rn[:, n : n + 1])
                tp = psum.tile([128, 128], F32, tag="pp")
                nc.tensor.transpose(tp, znsc, ident)
                j, k = divmod(n, CHUNK // 128)
                nc.vector.tensor_copy(znT[j][:, k * 128 : (k + 1) * 128], tp)

            # ---- main loop: sim rows x all columns ----------------------
            acc = persist.tile([128, NBLK, NCHUNK], F32)   # per-chunk exp sums
            diag_all = persist.tile([128, NBLK], F32)
            pos_all = persist.tile([128, NBLK], F32)

            for b in range(NBLK):
                lhsT = znT[0][:, b * 128 : (b + 1) * 128]  # block cols < 1024
                for c in range(NCHUNK):
                    pt = psum.tile([128, CHUNK], F32, tag="pp")
                    for s in range(CHUNK // SEG):
                        nc.tensor.matmul(
                            pt[:, s * SEG : (s + 1) * SEG],
                            lhsT,
                            znT[c][:, s * SEG : (s + 1) * SEG],
                            start=True,
                            stop=True,
                        )
                    if c == 0:  # diagonal at cols b*128..+128
                        scr = work.tile([128, 128], F32, tag="scr")
                        nc.vector.tensor_mul(scr, pt[:, b * 128 : b * 128 + 128], ident)
                        nc.vector.reduce_sum(diag_all[:, b : b + 1], scr, axis=AX.X)
                    if c == 2:  # positive at cols 4096 + b*128..+128
                        scr2 = work.tile([128, 128], F32, tag="scr2")
                        nc.vector.tensor_mul(scr2, pt[:, b * 128 : b * 128 + 128], ident)
                        nc.vector.reduce_sum(pos_all[:, b : b + 1], scr2, axis=AX.X)
                    ej = junkp.tile([128, CHUNK], F32, tag="ej")
                    nc.scalar.activation(
                        ej,
                        pt,
                        AF.Exp,
                        scale=INV_T,
                        bias=b_neg10,
                        accum_out=acc[:, b, c : c + 1],
                    )

            # ---- epilogue ----------------------------------------------
            sumexp = persist.tile([128, NBLK], F32)
            nc.vector.reduce_sum(sumexp, acc, axis=AX.X)      # [128,8,4] -> [128,8]
            e_diag = persist.tile([128, NBLK], F32)
            nc.scalar.activation(e_diag, diag_all, AF.Exp, scale=INV_T, bias=b_neg10)
            S = persist.tile([128, NBLK], F32)
            nc.vector.tensor_sub(S, sumexp, e_diag)
            lnS = persist.tile([128, NBLK], F32)
            nc.scalar.activation(lnS, S, AF.Ln, bias=b_zero)
            contrib = persist.tile([128, NBLK], F32)
            nc.vector.tensor_scalar_mul(contrib, pos_all, -INV_T)
            nc.vector.tensor_add(contrib, contrib, lnS)
            total = persist.tile([128, 1], F32)
            nc.vector.reduce_sum(total, contrib, axis=AX.X)
            nc.sync.dma_start(out=out[:, :], in_=total)

    nc.compile()
    return nc


_NC = None


def _get_nc() -> bass.Bass:
    global _NC
    if _NC is None:
        _NC = _build()
    return _NC


def kernel(emb0: np.ndarray, emb1: np.ndarray) -> np.ndarray:
    z = np.concatenate(
        [np.asarray(emb0, np.float32), np.asarray(emb1, np.float32)], axis=0
    )
    in_maps = [
        {"z": np.ascontiguousarray(np.roll(z, -c * ROWS, axis=0))}
        for c in range(NCORES)
    ]
    res = bass_utils.run_bass_kernel_spmd(_get_nc(), in_maps, core_ids=list(range(NCORES)))
    total = sum(float(r["partial"].sum(dtype=np.float64)) for r in res.results)
    return np.asarray(np.float32(INV_T + total / N))


# revision 8
# speedup vs baseline: 1.3191x; 1.3191x over previous
"""Trainium2 Bass kernel for nn_ContrastiveLoss (SimCLR-style, N=8192, D=128).

Sharding: rows of the NxN sim matrix split across 8 cores (1024 rows each).
Each core receives the full z = concat(emb0, emb1) ROTATED so its own rows
come first (np.roll(z, -core*1024, axis=0)).  With that rotation the diagonal
of row-block b sits at local columns [b*128, b*128+128) and the positive pair
at local columns [4096+b*128, ...), identical on every core -> one SPMD
program, no collectives.  Per-core output is a [128,1] partial sum of
(ln S_r - 10*sim_pos_r); the host combines: loss = 10 + sum(partials)/8192.

Math (per row r, fixed max = 1.0 since cosine sim <= 1):
  e_j  = exp(10*G_rj - 10),  S_r = sum_j e_j - e_rr
  lse_r = 10 + ln(S_r);  contrib_r = lse_r - 10*G_{r,pos(r)}
  loss  = mean_r(contrib_r)
"""

import sys

sys.path.insert(0, "/opt/trn_rl_repo")

from contextlib import ExitStack

import numpy as np

import concourse.bass as bass
import concourse.bacc as bacc
import concourse.tile as tile
from concourse import mybir
from concourse import bass_utils
from concourse.masks import make_identity

B = 4096
D = 128
N = 2 * B            # 8192 rows of z
NCORES = 8
ROWS = N // NCORES   # 1024 rows per core
NBLK = ROWS // 128   # 8 row-blocks per core
CHUNK = 2048         # psum tile width (4 banks)
NCHUNK = N // CHUNK  # 4 column chunks
SEG = 512            # matmul moving-operand max (fp32)
NTILE = N // 128     # 64 partition-tiles of z
INV_T = 10.0         # 1/temperature
EPS = 1e-8

F32 = mybir.dt.float32
BF16 = mybir.dt.bfloat16
AX = mybir.AxisListType
AF = mybir.ActivationFunctionType


def _build() -> bass.Bass:
    nc = bacc.Bacc(None)
    z_in = nc.declare_dram_parameter("z", [N, D], F32, isOutput=False)
    out = nc.declare_dram_parameter("partial", [128, 1], F32, isOutput=True)

    z_re = z_in.rearrange("(n p) d -> p n d", p=128)  # row = n*128 + p

    with tile.TileContext(nc) as tc:
        with ExitStack() as ctx:
            persist = ctx.enter_context(tc.tile_pool(name="persist", bufs=1))
            work = ctx.enter_context(tc.tile_pool(name="work", bufs=3))
            junkp = ctx.enter_context(tc.tile_pool(name="junk", bufs=2))
            psum = ctx.enter_context(tc.tile_pool(name="psum", bufs=2, space="PSUM"))

            ident = persist.tile([128, 128], F32)
            make_identity(nc, ident)
            # non-Copy activations need bias as an SBUF AP
            b_zero = persist.tile([128, 1], F32)
            nc.vector.memset(b_zero, 0.0)
            b_neg10 = persist.tile([128, 1], F32)
            nc.vector.memset(b_neg10, -INV_T)

            # ---- load z: [8192,128] -> SBUF [128p, 64, 128] -------------
            z_sb = persist.tile([128, NTILE, D], F32)
            DMA_G = 8  # tiles per DMA
            for i in range(NTILE // DMA_G):
                nc.sync.dma_start(
                    out=z_sb[:, i * DMA_G : (i + 1) * DMA_G, :],
                    in_=z_re[:, i * DMA_G : (i + 1) * DMA_G, :],
                )

            # ---- row norms ----------------------------------------------
            sq = persist.tile([128, NTILE, D], F32)
            sumsq = persist.tile([128, NTILE], F32)
            for i in range(NTILE // DMA_G):
                sl = slice(i * DMA_G, (i + 1) * DMA_G)
                nc.vector.tensor_mul(sq[:, sl, :], z_sb[:, sl, :], z_sb[:, sl, :])
                nc.vector.reduce_sum(sumsq[:, sl], sq[:, sl, :], axis=AX.X)
            rn = persist.tile([128, NTILE], F32)
            nc.scalar.activation(rn, sumsq, AF.Sqrt, bias=b_zero)          # norm
            nc.vector.tensor_scalar_max(rn, rn, EPS)          # clamp
            nc.vector.reciprocal(rn, rn)                      # 1/max(norm,eps)

            # ---- normalize + transpose into znT chunks [128d, 2048] -----
            znT = [
                persist.tile([128, CHUNK], BF16, tag=f"znT{j}", name=f"znT{j}")
                for j in range(NCHUNK)
            ]
            for n in range(NTILE):
                znsc = work.tile([128, 128], F32, tag="znsc")
                nc.vector.tensor_scalar_mul(znsc, z_sb[:, n, :], rn[:, n : n + 1])
                tp = psum.tile([128, 128], F32, tag="pp")
                nc.tensor.transpose(tp, znsc, ident)
                j, k = divmod(n, CHUNK // 128)
                nc.vector.tensor_copy(znT[j][:, k * 128 : (k + 1) * 128], tp)

            # ---- main loop: sim rows x all columns ----------------------
            acc = persist.tile([128, NBLK, NCHUNK], F32)   # per-chunk exp sums
            diag_all = persist.tile([128, NBLK], F32)
            pos_all = persist.tile([128, NBLK], F32)

            for b in range(NBLK):
                lhsT = znT[0][:, b * 128 : (b + 1) * 128]  # block cols < 1024
                for c in range(NCHUNK):
                    pt = psum.tile([128, CHUNK], F32, tag="pp")
                    for s in range(CHUNK // SEG):
                        nc.tensor.matmul(
                            pt[:, s * SEG : (s + 1) * SEG],
                            lhsT,
                            znT[c][:, s * SEG : (s + 1) * SEG],
                            start=True,
                            stop=True,
                        )
                    if c == 0:  # diagonal at cols b*128..+128
                        scr = work.tile([128, 128], F32, tag="scr")
                        nc.vector.tensor_mul(scr, pt[:, b * 128 : b * 128 + 128], ident)
                        nc.vector.reduce_sum(diag_all[:, b : b + 1], scr, axis=AX.X)
                    if c == 2:  # positive at cols 4096 + b*128..+128
                        scr2 = work.tile([128, 128], F32, tag="scr2")
                        nc.vector.tensor_mul(scr2, pt[:, b * 128 : b * 128 + 128], ident)
                        nc.vector.reduce_sum(pos_all[:, b : b + 1], scr2, axis=AX.X)
                    ej = junkp.tile([128, CHUNK], F32, tag="ej")
                    nc.scalar.activation(
                        ej,
                        pt,
                        AF.Exp,
                        scale=INV_T,
                        bias=b_neg10,
                        accum_out=acc[:, b, c : c + 1],
                    )

            # ---- epilogue ----------------------------------------------
            sumexp = persist.tile([128, NBLK], F32)
            nc.vector.reduce_sum(sumexp, acc, axis=AX.X)      # [128,8,4] -> [128,8]
            e_diag = persist.tile([128, NBLK], F32)
            nc.scalar.activation(e_diag, diag_all, AF.Exp, scale=INV_T, bias=b_neg10)
            S = persist.tile([128, NBLK], F32)
            nc.vector.tensor_sub(S, sumexp, e_diag)
            lnS = persist.tile([128, NBLK], F32)
            nc.scalar.activation(lnS, S, AF.Ln, bias=b_zero)
            contrib = persist.tile([128, NBLK], F32)
            nc.vector.tensor_scalar_mul(contrib, pos_all, -INV_T)
            nc.vector.tensor_add(contrib, contrib, lnS)
            total = persist.tile([128, 1], F32)
            nc.vector.reduce_sum(total, contrib, axis=AX.X)
            nc.sync.dma_start(out=out[:, :], in_=total)

    nc.compile()
    return nc


_NC = None


def _get_nc() -> bass.Bass:
    global _NC
    if _NC is None:
        _NC = _build()
    return _NC


def kernel(emb0: np.ndarray, emb1: np.ndarray) -> np.ndarray:
    z = np.concatenate(
        [np.asarray(emb0, np.float32), np.asarray(emb1, np.float32)], axis=0
    )
    in_maps = [
        {"z": np.ascontiguousarray(np.roll(z, -c * ROWS, axis=0))}
        for c in range(NCORES)
    ]
    res = bass_utils.run_bass_kernel_spmd(_get_nc(), in_maps, core_ids=list(range(NCORES)))
    total = sum(float(r["partial"].sum(dtype=np.float64)) for r in res.results)
    return np.asarray(np.float32(INV_T + total / N))
